# revision 1
# baseline (speedup 1.0000x reference)
"""Fused transformer block (LN1 -> causal MHA -> residual -> LN2 -> FFN -> residual)
for Trainium2, distributed over 8 NeuronCores by sequence sharding.

Sharding: core c handles batch c//4, query tokens [ (c%4)*512, (c%4+1)*512 ).
Every core receives its batch's tokens ROTATED so that its query chunk sits at
positions 0:512 -- this keeps the program identical across cores (SPMD); the
causal structure is carried by a per-core additive mask tensor (host data).
Matmuls run in bf16 with fp32 PSUM accumulation; LN/softmax math in fp32.
"""

import os
import sys

import numpy as np

if "/opt/trn_rl_repo" not in sys.path:
    sys.path.insert(0, "/opt/trn_rl_repo")

import ml_dtypes

B, T, D = 2, 2048, 1024
H, HS = 16, 64
F = 4 * D
TQ = 512          # query tokens per core
NCORES = 8
EPS = 1e-5
NEG = -1e9

BF16 = ml_dtypes.bfloat16

_CACHE = {}


def _build(flags):
    """Build the Bass program (same for all cores). flags: (has_bo, has_b2)."""
    import concourse.bass as bass
    import concourse.mybir as mybir
    import concourse.tile as tile
    from concourse import bacc
    from concourse.bass import ts
    from concourse.masks import make_identity

    has_bo, has_b2 = flags
    f32 = mybir.dt.float32
    bf16 = mybir.dt.bfloat16
    Alu = mybir.AluOpType
    Act = mybir.ActivationFunctionType

    nc = bacc.Bacc("TRN2", target_bir_lowering=False, debug=False, num_devices=1)

    # ---- DRAM I/O ----
    x_kv = nc.dram_tensor("x_kv", [T, D], bf16, kind="ExternalInput").ap()
    x_q = nc.dram_tensor("x_q", [TQ, D], f32, kind="ExternalInput").ap()
    maskD = nc.dram_tensor("maskD", [TQ, 2 * TQ], bf16, kind="ExternalInput").ap()
    blkb = nc.dram_tensor("blkb", [T // 128], f32, kind="ExternalInput").ap()
    wq = nc.dram_tensor("wq", [D, D], bf16, kind="ExternalInput").ap()
    wk = nc.dram_tensor("wk", [D, D], bf16, kind="ExternalInput").ap()
    wv = nc.dram_tensor("wv", [D, D], bf16, kind="ExternalInput").ap()
    wo = nc.dram_tensor("wo", [D, D], bf16, kind="ExternalInput").ap()
    w1 = nc.dram_tensor("w1", [D, F], bf16, kind="ExternalInput").ap()
    w2 = nc.dram_tensor("w2", [F, D], bf16, kind="ExternalInput").ap()
    b1d = nc.dram_tensor("b1", [F], f32, kind="ExternalInput").ap()
    bod = nc.dram_tensor("bo", [D], f32, kind="ExternalInput").ap() if has_bo else None
    b2d = nc.dram_tensor("b2", [D], f32, kind="ExternalInput").ap() if has_b2 else None
    out = nc.dram_tensor("out", [TQ, D], f32, kind="ExternalOutput").ap()

    KT = T // 128      # 16 token tiles of kv
    DC = D // 128      # 8 feature chunks
    FC = F // 128      # 32 hidden chunks
    QS = TQ // 128     # 4 query subtiles

    with tile.TileContext(nc) as tc:
        with (
            tc.tile_pool(name="const", bufs=1) as cst,
            tc.tile_pool(name="actB", bufs=1) as actB,
        ):
            # --- constants (eager) ---
            ident = cst.tile([128, 128], bf16)
            make_identity(nc, ident)
            eps_t = cst.tile([128, 1], f32)
            nc.vector.memset(eps_t, EPS)
            b1_sb = cst.tile([128, FC], f32)
            nc.scalar.dma_start(out=b1_sb, in_=b1d.rearrange("(m p) -> p m", p=128))
            if has_bo:
                bo_b = cst.tile([128, D], f32)
                nc.scalar.dma_start(
                    out=bo_b,
                    in_=bass.AP(tensor=bod.tensor, offset=bod.offset,
                                ap=[[0, 128]] + list(bod.ap)))
            if has_b2:
                b2_b = cst.tile([128, D], f32)
                nc.scalar.dma_start(
                    out=b2_b,
                    in_=bass.AP(tensor=b2d.tensor, offset=b2d.offset,
                                ap=[[0, 128]] + list(b2d.ap)))
            mask_sb = cst.tile([128, QS, 2 * TQ], bf16)
            nc.scalar.dma_start(
                out=mask_sb, in_=maskD.rearrange("(k p) q -> p k q", p=128))
            blk_sb = cst.tile([128, KT], f32)
            nc.scalar.dma_start(
                out=blk_sb,
                in_=bass.AP(tensor=blkb.tensor, offset=blkb.offset,
                            ap=[[0, 128]] + list(blkb.ap)))

            # --- persistent activations (eager alloc, written later) ---
            q_fm = [actB.tile([128, TQ], bf16, name=f"qfm{m}") for m in range(DC)]
            k_fm = [actB.tile([128, T], bf16, name=f"kfm{m}") for m in range(DC)]
            v_sb = [actB.tile([128, H, HS + 1], bf16, name=f"vsb{t}")
                    for t in range(KT)]
            attnT = [actB.tile([128, TQ], bf16, name=f"at{d}") for d in range(DC)]
            x2_sb = [actB.tile([128, D], f32, name=f"x2{i}") for i in range(QS)]
            h2_fm = [actB.tile([128, TQ], bf16, name=f"h2f{d}") for d in range(DC)]

            # ================= Phase 1+2: LN1 + transpose ==================
            with tc.tile_pool(name="hfmP", bufs=1) as hfmP:
              h_fm = [hfmP.tile([128, T], bf16, name=f"hfm{d}")
                      for d in range(DC)]
              with (
                tc.tile_pool(name="ph12", bufs=3) as ph12,
                tc.tile_pool(name="wvP", bufs=1) as wvP,
                tc.tile_pool(name="qwP", bufs=2) as qwP,
                tc.tile_pool(name="psT", bufs=4, space="PSUM") as psT,
                tc.tile_pool(name="psV", bufs=3, space="PSUM") as psV,
              ):
                wvcs = []
                for n in range(2):
                    wvc = wvP.tile([128, DC, 512], bf16, tag=f"wvc{n}",
                                   name=f"wvc{n}")
                    nc.sync.dma_start(
                        out=wvc,
                        in_=wv[:, ts(n, 512)].rearrange("(k p) c -> p k c", p=128))
                    wvcs.append(wvc)
                for t in range(KT):
                    nc.vector.memset(v_sb[t][:, :, HS:HS + 1], 1.0)
                for t in range(KT):
                    xt = ph12.tile([128, D], bf16, tag="xt", name="xt")
                    nc.sync.dma_start(out=xt, in_=x_kv[ts(t, 128), :])
                    xg = xt.rearrange("p (n f) -> p n f", f=512)
                    stats = ph12.tile([128, 2, 6], f32, tag="st", name="st")
                    for sg in range(2):
                        nc.vector.bn_stats(out=stats[:, sg, :], in_=xg[:, sg, :])
                    mv = ph12.tile([128, 2], f32, tag="mv", name="mv")
                    nc.vector.bn_aggr(out=mv, in_=stats)
                    rstd = ph12.tile([128, 1], f32, tag="rs", name="rs")
                    nc.scalar.activation(out=rstd, in_=mv[:, 1:2], func=Act.Sqrt,
                                         bias=eps_t, scale=1.0)
                    nc.vector.reciprocal(out=rstd, in_=rstd)
                    ht = ph12.tile([128, D], bf16, tag="ht", name="ht")
                    nc.vector.tensor_scalar(
                        out=ht, in0=xt, scalar1=mv[:, 0:1], scalar2=rstd,
                        op0=Alu.subtract, op1=Alu.mult)
                    for d in range(DC):
                        ps = psT.tile([128, 128], bf16, tag="tr", name="tr")
                        nc.tensor.transpose(ps, ht[:, ts(d, 128)], ident)
                        nc.vector.tensor_copy(out=h_fm[d][:, ts(t, 128)], in_=ps)
                    # V projection for this token tile (fills PE under LN DVE)
                    pvv = [psV.tile([128, 512], f32, tag="mm", name="psv")
                           for n in range(2)]
                    for k in range(DC):
                        for n in range(2):
                            nc.tensor.matmul(pvv[n], h_fm[k][:, ts(t, 128)],
                                             wvcs[n][:, k, :],
                                             start=(k == 0), stop=(k == DC - 1))
                    for n in range(2):
                        nc.vector.tensor_copy(
                            out=v_sb[t][:, ts(n, 8), 0:HS],
                            in_=pvv[n].rearrange("p (h d) -> p h d", d=HS))
                    if t == 3:
                        # Q projection (needs only tiles 0..3 of h_fm)
                        for m in range(DC):
                            qwc = qwP.tile([128, DC, 128], bf16, tag="qwc",
                                           name="qwc")
                            nc.sync.dma_start(
                                out=qwc,
                                in_=wq[:, ts(m, 128)].rearrange(
                                    "(k p) c -> p k c", p=128))
                            psq = psV.tile([128, TQ], f32, tag="mm", name="psq")
                            for k in range(DC):
                                nc.tensor.matmul(psq, qwc[:, k, :],
                                                 h_fm[k][:, 0:TQ],
                                                 start=(k == 0),
                                                 stop=(k == DC - 1))
                            nc.vector.tensor_copy(out=q_fm[m], in_=psq)

              # ================= Phase 3: QKV projections ================
              with (
                  tc.tile_pool(name="wst", bufs=3) as wst,
                  tc.tile_pool(name="psM", bufs=6, space="PSUM") as psM,
              ):
                  # K (k outer, 4 live psums: lhsT loaded once per k)
                  for m in range(DC):
                      wc = wst.tile([128, DC, 128], bf16, tag="wcol", name="wc")
                      nc.sync.dma_start(
                          out=wc,
                          in_=wk[:, ts(m, 128)].rearrange("(k p) c -> p k c", p=128))
                      pss = [psM.tile([128, 512], f32, tag="mm", name="psk")
                             for n in range(T // 512)]
                      for k in range(DC):
                          for n in range(T // 512):
                              nc.tensor.matmul(pss[n], wc[:, k, :],
                                               h_fm[k][:, ts(n, 512)],
                                               start=(k == 0), stop=(k == DC - 1))
                      for n in range(T // 512):
                          if n % 2 == 0:
                              nc.vector.tensor_copy(out=k_fm[m][:, ts(n, 512)],
                                                    in_=pss[n])
                          else:
                              nc.scalar.copy(out=k_fm[m][:, ts(n, 512)],
                                             in_=pss[n])

            # ================= Phase 4: attention =======================
            with (
                tc.tile_pool(name="ph4", bufs=8) as ph4,
                tc.tile_pool(name="smm", bufs=3) as smm,
                tc.tile_pool(name="psS", bufs=3, space="PSUM") as psS,
                tc.tile_pool(name="psAV", bufs=2, space="PSUM") as psAV,
            ):
                LAG = 6

                def emit_av(ent):
                    kq, pav0, pav1, pk, first, last, pe = ent
                    h0, h1 = 2 * kq, 2 * kq + 1
                    nc.tensor.matmul(pav0, v_sb[pk][:, h0, :], pe[:, 0:TQ],
                                     start=first, stop=last)
                    nc.tensor.matmul(pav1, v_sb[pk][:, h1, :],
                                     pe[:, TQ:2 * TQ],
                                     start=first, stop=last)
                    if last:
                        for sub, pav in ((0, pav0), (1, pav1)):
                            ro = sub * HS
                            # fast raw evac releases the PSUM bank; the
                            # normalize chain then runs from SBUF off the
                            # AV critical path
                            raw = smm.tile([HS + 1, TQ], f32, tag="raw",
                                           name="raw")
                            nc.vector.tensor_copy(out=raw, in_=pav)
                            recip = smm.tile([1, TQ], f32, tag="recip",
                                             name="recip")
                            nc.vector.reciprocal(out=recip,
                                                 in_=raw[HS:HS + 1, :])
                            bcast = smm.tile([HS, TQ], f32, tag="bcast",
                                             name="bcast")
                            nc.gpsimd.partition_broadcast(bcast, recip)
                            nc.vector.tensor_tensor(
                                out=attnT[kq][ro:ro + HS, :],
                                in0=raw[0:HS, :], in1=bcast, op=Alu.mult)

                pending = []
                for kq in range(H // 2):
                    pav0 = psAV.tile([HS + 1, TQ], f32, tag="av", name="pav0")
                    pav1 = psAV.tile([HS + 1, TQ], f32, tag="av", name="pav1")
                    for kb in list(range(QS, KT)) + list(range(QS)):
                        first, last_kb = kb == QS, kb == QS - 1
                        pss = psS.tile([128, 2 * TQ], f32, tag="s", name="pss")
                        for sub in range(2):
                            ro = sub * HS
                            nc.tensor.matmul(
                                pss[:, ts(sub, TQ)],
                                k_fm[kq][ro:ro + HS, ts(kb, 128)],
                                q_fm[kq][ro:ro + HS, 0:TQ],
                                start=True, stop=True)
                        if kb < QS:
                            for sub in range(2):
                                nc.vector.tensor_tensor(
                                    out=pss[:, ts(sub, TQ)],
                                    in0=pss[:, ts(sub, TQ)],
                                    in1=mask_sb[:, kb, ts(sub, TQ)],
                                    op=Alu.add)
                        et = ph4.tile([128, 2 * TQ], bf16, tag="exp", name="et")
                        nc.scalar.activation(
                            out=et, in_=pss, func=Act.Exp, scale=0.125,
                            bias=blk_sb[:, kb:kb + 1])
                        pending.append((kq, pav0, pav1, kb, first, last_kb, et))
                        if len(pending) > LAG:
                            emit_av(pending.pop(0))
                for ent in pending:
                    emit_av(ent)

            # ========= Phase 5+6: Wo + residual + LN2 + transpose =========
            with (
                tc.tile_pool(name="ph5", bufs=2) as ph5,
                tc.tile_pool(name="ph6", bufs=2) as ph6,
                tc.tile_pool(name="psO", bufs=4, space="PSUM") as psO,
                tc.tile_pool(name="psT2", bufs=4, space="PSUM") as psT2,
            ):
                xq_sb = [ph5.tile([128, D], f32, tag=f"xq{i}", name=f"xq{i}")
                         for i in range(QS)]
                for i in range(QS):
                    nc.sync.dma_start(out=xq_sb[i], in_=x_q[ts(i, 128), :])
                wocs = []
                for n in range(2):
                    woc = ph5.tile([128, DC, 512], bf16, tag=f"woc{n}",
                                   name=f"woc{n}")
                    nc.sync.dma_start(
                        out=woc,
                        in_=wo[:, ts(n, 512)].rearrange("(k p) c -> p k c", p=128))
                    wocs.append(woc)
                for i in range(QS):
                    pss = [psO.tile([128, 512], f32, tag="o", name="pso")
                           for n in range(2)]
                    for k in range(DC):
                        for n in range(2):
                            nc.tensor.matmul(pss[n], attnT[k][:, ts(i, 128)],
                                             wocs[n][:, k, :],
                                             start=(k == 0), stop=(k == DC - 1))
                    for n in range(2):
                        nc.vector.tensor_tensor(
                            out=x2_sb[i][:, ts(n, 512)], in0=pss[n],
                            in1=xq_sb[i][:, ts(n, 512)], op=Alu.add)
                    if has_bo:
                        nc.vector.tensor_tensor(
                            out=x2_sb[i], in0=x2_sb[i], in1=bo_b, op=Alu.add)
                    # LN2 + transpose for this subtile, interleaved with the
                    # next subtile's Wo matmuls
                    xg = x2_sb[i].rearrange("p (n f) -> p n f", f=512)
                    stats = ph6.tile([128, 2, 6], f32, tag="st", name="st6")
                    for sg in range(2):
                        nc.vector.bn_stats(out=stats[:, sg, :], in_=xg[:, sg, :])
                    mv = ph6.tile([128, 2], f32, tag="mv", name="mv6")
                    nc.vector.bn_aggr(out=mv, in_=stats)
                    rstd = ph6.tile([128, 1], f32, tag="rs", name="rs6")
                    nc.scalar.activation(out=rstd, in_=mv[:, 1:2], func=Act.Sqrt,
                                         bias=eps_t, scale=1.0)
                    nc.vector.reciprocal(out=rstd, in_=rstd)
                    h2t = ph6.tile([128, D], bf16, tag="h2t", name="h2t")
                    nc.vector.tensor_scalar(
                        out=h2t, in0=x2_sb[i], scalar1=mv[:, 0:1], scalar2=rstd,
                        op0=Alu.subtract, op1=Alu.mult)
                    for d in range(DC):
                        ps = psT2.tile([128, 128], bf16, tag="tr", name="tr2")
                        nc.tensor.transpose(ps, h2t[:, ts(d, 128)], ident)
                        nc.vector.tensor_copy(out=h2_fm[d][:, ts(i, 128)], in_=ps)

            # ================= Phase 7: FFN1 ============================
            with tc.tile_pool(name="g1P", bufs=1) as g1P:
                g1 = [g1P.tile([128, TQ], bf16, name=f"g1t{m}") for m in range(FC)]
                with (
                    tc.tile_pool(name="ph7", bufs=3) as ph7,
                    tc.tile_pool(name="ph8", bufs=4) as ph8,
                    tc.tile_pool(name="psF", bufs=2, space="PSUM") as psF,
                    tc.tile_pool(name="ps8", bufs=1, space="PSUM") as ps8,
                ):
                    def ffn2_sweep(m, psum2, ilist, w2c):
                        for i in ilist:
                            for n in range(2):
                                nc.tensor.matmul(
                                    psum2[(i % 2) * 2 + n],
                                    g1[m][:, ts(i, 128)],
                                    w2c[:, ts(n, 512)],
                                    start=(m == 0), stop=(m == FC - 1))

                    def ffn2_evac(psum2, ilist):
                        for i in ilist:
                            ot = ph8.tile([128, D], f32, tag="ot", name="ot")
                            for n in range(2):
                                nc.vector.tensor_tensor(
                                    out=ot[:, ts(n, 512)],
                                    in0=psum2[(i % 2) * 2 + n],
                                    in1=x2_sb[i][:, ts(n, 512)], op=Alu.add)
                            if has_b2:
                                nc.vector.tensor_tensor(
                                    out=ot, in0=ot, in1=b2_b, op=Alu.add)
                            nc.sync.dma_start(out=out[ts(i, 128), :], in_=ot)

                    # FFN1 + FFN2 sweep A (i=0,1) interleaved over m
                    psum2a = [ps8.tile([128, 512], f32, tag=f"p8_{j}",
                                       name=f"p8a{j}") for j in range(4)]
                    for m in range(FC):
                        wc = ph7.tile([128, DC, 128], bf16, tag="w1c", name="w1c")
                        nc.sync.dma_start(
                            out=wc,
                            in_=w1[:, ts(m, 128)].rearrange("(k p) c -> p k c", p=128))
                        ps = psF.tile([128, TQ], f32, tag="mm", name="psf")
                        for k in range(DC):
                            nc.tensor.matmul(ps, wc[:, k, :], h2_fm[k][:, 0:TQ],
                                             start=(k == 0), stop=(k == DC - 1))
                        nc.scalar.activation(out=g1[m], in_=ps, func=Act.Relu,
                                             bias=b1_sb[:, m:m + 1], scale=1.0)
                        w2c = ph8.tile([128, D], bf16, tag="w2c", name="w2c")
                        nc.sync.dma_start(out=w2c, in_=w2[ts(m, 128), :])
                        ffn2_sweep(m, psum2a, (0, 1), w2c)
                    ffn2_evac(psum2a, (0, 1))

                    # FFN2 sweep B (i=2,3): W2 streamed a second time
                    psum2b = [ps8.tile([128, 512], f32, tag=f"p8_{j}",
                                       name=f"p8b{j}") for j in range(4)]
                    for m in range(FC):
                        w2c = ph8.tile([128, D], bf16, tag="w2c", name="w2c")
                        nc.sync.dma_start(out=w2c, in_=w2[ts(m, 128), :])
                        ffn2_sweep(m, psum2b, (2, 3), w2c)
                    ffn2_evac(psum2b, (2, 3))

    nc.compile()
    return nc


def _prep(inputs):
    """Host-side shard prep. Returns in_maps (one dict per core)."""
    x = np.asarray(inputs["x"], np.float32)
    ln1_g = np.asarray(inputs["ln1_g"], np.float32)
    ln1_b = np.asarray(inputs["ln1_b"], np.float32)
    ln2_g = np.asarray(inputs["ln2_g"], np.float32)
    ln2_b = np.asarray(inputs["ln2_b"], np.float32)
    assert np.all(ln1_b == 0.0) and np.all(ln2_b == 0.0), "ln biases must be 0"

    # fold ln gains into the consuming weight matrices
    wq = (ln1_g[:, None] * np.asarray(inputs["Wq"], np.float32)).astype(BF16)
    wk = (ln1_g[:, None] * np.asarray(inputs["Wk"], np.float32)).astype(BF16)
    wv = (ln1_g[:, None] * np.asarray(inputs["Wv"], np.float32)).astype(BF16)
    wo = np.asarray(inputs["Wo"], np.float32).astype(BF16)
    w1 = (ln2_g[:, None] * np.asarray(inputs["W1"], np.float32)).astype(BF16)
    w2 = np.asarray(inputs["W2"], np.float32).astype(BF16)
    b1 = np.ascontiguousarray(np.asarray(inputs["b1"], np.float32))
    bo = np.asarray(inputs["bo"], np.float32)
    b2 = np.asarray(inputs["b2"], np.float32)
    has_bo = bool(np.any(bo != 0.0))
    has_b2 = bool(np.any(b2 != 0.0))

    xb = x.astype(BF16)
    qidx = np.arange(TQ)
    in_maps = []
    for c in range(NCORES):
        b, j = divmod(c, B * 2)  # 4 chunks per batch
        b, j = c // 4, c % 4
        q0 = j * TQ
        perm = np.concatenate([
            np.arange(q0, q0 + TQ),
            np.arange(0, q0),
            np.arange(q0 + TQ, T),
        ])
        x_kv = np.ascontiguousarray(xb[b][perm])
        x_q = np.ascontiguousarray(x[b, q0:q0 + TQ])
        maskD = np.where(qidx[:, None] <= qidx[None, :], np.float32(0.0),
                         np.float32(NEG)).astype(BF16)
        maskD = np.concatenate([maskD, maskD], axis=1)
        blkb = np.zeros((T // 128,), np.float32)
        for kb in range(4, T // 128):
            if kb * 128 >= 512 + q0:
                blkb[kb] = NEG
        m = {
            "x_kv": x_kv, "x_q": x_q,
            "maskD": np.ascontiguousarray(maskD), "blkb": blkb,
            "wq": wq, "wk": wk, "wv": wv, "wo": wo,
            "w1": w1, "w2": w2, "b1": b1,
        }
        if has_bo:
            m["bo"] = bo
        if has_b2:
            m["b2"] = b2
        in_maps.append(m)
    return in_maps, (has_bo, has_b2)


def _run(inputs, profile_dir=None):
    from concourse import bass_utils

    in_maps, flags = _prep(inputs)
    if flags not in _CACHE:
        _CACHE[flags] = _build(flags)
    nc = _CACHE[flags]

    if profile_dir is not None:
        from concourse import bass2jax
        from trn_agent_boot.trn_boot import _ntff_profile_via_ctypes
        hook = _ntff_profile_via_ctypes("/opt/axon/libaxon_pjrt.so")
        with hook(profile_dir, [0]):
            results = bass2jax.run_bass_via_pjrt(nc, in_maps, n_cores=NCORES)
    else:
        res = bass_utils.run_bass_kernel_spmd(
            nc, in_maps, core_ids=list(range(NCORES))
        )
        results = res.results

    out = np.empty((B, T, D), np.float32)
    for c in range(NCORES):
        b, j = c // 4, c % 4
        out[b, j * TQ:(j + 1) * TQ] = results[c]["out"]
    return out


def kernel(**inputs) -> np.ndarray:
    return _run(inputs)

